# revision 45
# baseline (speedup 1.0000x reference)
"""Trainium2 Bass kernel for nn_LocalAttention (block-local sliding-window attention
with BitLinear projections and a pre-LayerNorm).

Sharding: 8 cores = B(4) x T-halves(2). Each core processes 2048 own tokens plus a
256-token halo (previous block's k/v); halo for the first half of each batch is
zero-padded and masked out (matches the reference's zero-block + validity mask).

Device pipeline per core (SPMD, per-core data differs):
  LN -> (DMA-xbar transpose) hT -> QKV matmuls (bf16 ternary weights, scales
  applied in the PSUM->SBUF copy) -> block-local attention with transposed
  scores [kj, qi] (mask applied multiplicatively post-exp; softmax denominator
  via an appended ones-column on V; normalization via gpsimd partition-broadcast
  of the reciprocal row) -> output projection (scales folded into the weight on
  host) streamed straight from PSUM to HBM.
"""

import sys

import numpy as np
import ml_dtypes

sys.path.insert(0, "/opt/trn_rl_repo")

import concourse.bass as bass  # noqa: E402
import concourse.tile as tile  # noqa: E402
from concourse import bacc, mybir  # noqa: E402
from concourse.bass import ts  # noqa: E402
from concourse.bass_utils import run_bass_kernel_spmd  # noqa: E402

B, T, D = 4, 4096, 1024
H, DH = 16, 64
WIN = 256
EPS = 1e-5
NCORES = 8
OWN = T // 2          # own tokens per core (2048)
HALO = WIN            # halo tokens (256)
NTOK = OWN + HALO     # local token rows (2304)
CHUNK = 512           # own tokens per main chunk
NCHUNK = OWN // CHUNK # 4
KT = D // 128         # 8 contraction tiles
BF16 = mybir.dt.bfloat16
F32 = mybir.dt.float32

_CACHE = {}


def _quantize(w):
    scale = np.clip(np.mean(np.abs(w), axis=1, keepdims=True), 1e-5, None)
    w_q = np.clip(np.round(w / scale), -1.0, 1.0)
    return w_q.astype(np.float32), scale[:, 0].astype(np.float32)


def _build_masks():
    # transposed masks maskT[j, i]: window index j in [0, 2*WIN), query i in [0, WIN)
    J = np.arange(2 * WIN)[:, None]
    I = np.arange(WIN)[None, :]
    rest = ((I < J) & (I >= J - WIN)).astype(np.float32)
    first = rest * (J >= WIN)
    # device layout [128, 4, 256]: row p of tile jt is j = jt*128 + p
    def dev(m):
        return m.reshape(4, 128, WIN).transpose(1, 0, 2).reshape(128, 4 * WIN)
    return dev(first).astype(ml_dtypes.bfloat16), dev(rest).astype(ml_dtypes.bfloat16)


def _build_bass():
    nc = bacc.Bacc("TRN2", target_bir_lowering=False, debug=False,
                   enable_asserts=False, num_devices=NCORES)
    ap_x = nc.dram_tensor("x_shard", [NTOK, D], BF16, kind="ExternalInput").ap()
    ap_wq = nc.dram_tensor("wqkvT", [D, 3 * H * DH], BF16, kind="ExternalInput").ap()
    ap_wo = nc.dram_tensor("woutT", [H * DH, D], BF16, kind="ExternalInput").ap()
    ap_sc = nc.dram_tensor("sc_qk", [128, 16], F32, kind="ExternalInput").ap()
    ap_mask = nc.dram_tensor("masks", [2, 128, 4 * WIN], BF16, kind="ExternalInput").ap()
    ap_y = nc.dram_tensor("y_shard", [OWN, D], F32, kind="ExternalOutput").ap()

    with tile.TileContext(nc) as tc:
        _emit(tc, ap_x, ap_wq, ap_wo, ap_sc, ap_mask, ap_y)
    nc.compile()
    return nc


def _emit(tc, ap_x, ap_wq, ap_wo, ap_sc, ap_mask, ap_y):
    import contextlib
    with contextlib.ExitStack() as ctx:
        _emit_body(tc, ctx, ap_x, ap_wq, ap_wo, ap_sc, ap_mask, ap_y)


def _emit_body(tc, ctx, ap_x, ap_wq, ap_wo, ap_sc, ap_mask, ap_y):
    nc = tc.nc
    const = ctx.enter_context(tc.tile_pool(name="const", bufs=1))
    xp = ctx.enter_context(tc.tile_pool(name="xp", bufs=2))
    lnp = ctx.enter_context(tc.tile_pool(name="lnp", bufs=4))
    hp = ctx.enter_context(tc.tile_pool(name="hp", bufs=2))
    hTp = ctx.enter_context(tc.tile_pool(name="hTp", bufs=2))
    qTp = ctx.enter_context(tc.tile_pool(name="qTp", bufs=2))
    kTp = ctx.enter_context(tc.tile_pool(name="kTp", bufs=3))
    vp = ctx.enter_context(tc.tile_pool(name="vp", bufs=3))
    ep = ctx.enter_context(tc.tile_pool(name="ep", bufs=6))
    atp = ctx.enter_context(tc.tile_pool(name="atp", bufs=2))
    op = ctx.enter_context(tc.tile_pool(name="op", bufs=2))
    rp = ctx.enter_context(tc.tile_pool(name="rp", bufs=2))
    ps_sc = ctx.enter_context(tc.tile_pool(name="ps_sc", bufs=2, space="PSUM"))
    ps_mm = ctx.enter_context(tc.tile_pool(name="ps_mm", bufs=2, space="PSUM"))

    # ---- small constants ----
    sc_sb = const.tile([128, 16], F32)
    nc.gpsimd.dma_start(sc_sb[:], ap_sc)
    mask_sb = const.tile([128, 2, 4 * WIN], BF16)
    nc.gpsimd.dma_start(mask_sb[:], ap_mask.rearrange("m p w -> p m w"))
    eps_sb = const.tile([128, 1], F32)
    nc.vector.memset(eps_sb[:], EPS)
    # weight tiles (DMAs emitted after the first LN batch, see below)
    wq_g = []
    for g in range(4):
        wqt = const.tile([128, 2, 3 * H * DH], BF16, tag=f"wqg{g}")
        wq_g.append(wqt)
    wq_sbk = [wq_g[k // 2][:, k % 2, :] for k in range(KT)]
    wo_g = []
    for g in range(2):
        wot = const.tile([128, 4, D], BF16, tag=f"wog{g}")
        wo_g.append(wot)
    wo_sbk = [wo_g[k // 4][:, k % 4, :] for k in range(KT)]

    def emit_weight_dmas():
        # one DMA per 128-row k-tile: finer grain lets latency-critical DMAs
        # (x loads, h transposes) slip between weight pieces on the DMA engines
        for k in range(KT):
            nc.gpsimd.dma_start(wq_g[k // 2][:, k % 2, :],
                                ap_wq[k * 128:(k + 1) * 128, :])

    def emit_wo_dmas():
        # wo is first needed by outproj(0), much later: keep it off the
        # startup DMA critical path
        for k in range(0, KT, 2):
            nc.gpsimd.dma_start(wo_g[k // 4][:, k % 4: k % 4 + 2, :],
                                ap_wo[k * 128:(k + 2) * 128, :]
                                .rearrange("(k p) o -> p k o", p=128))

    def load_x(row0, ntile):
        xts = []
        for i2 in range(ntile // 2):
            xt = xp.tile([128, 2, D], BF16, tag="xt")
            nc.sync.dma_start(xt[:], ap_x[row0 + i2 * 256: row0 + (i2 + 1) * 256, :]
                              .rearrange("(i p) d -> p i d", p=128))
            xts.append(xt)
        return xts

    def ln_transpose(hT, xts):
        # fills hT[:, i, :, :] for each 128-token tile of the loaded x batch
        for i2, xt in enumerate(xts):
            ht2 = hp.tile([128, 2, D], BF16, tag="ht")
            for j in range(2):
                st = lnp.tile([128, 2, 6], F32, tag="st")
                nc.vector.bn_stats(out=st[:, 0, :], in_=xt[:, j, 0:512])
                nc.vector.bn_stats(out=st[:, 1, :], in_=xt[:, j, 512:1024])
                mv = lnp.tile([128, 2], F32, tag="mv")
                nc.vector.bn_aggr(out=mv[:], in_=st[:])
                sd = lnp.tile([128, 1], F32, tag="sd")
                nc.scalar.activation(out=sd[:], in_=mv[:, 1:2],
                                     func=mybir.ActivationFunctionType.Sqrt,
                                     bias=eps_sb[:], scale=1.0)
                rs = lnp.tile([128, 1], F32, tag="rs")
                nc.vector.reciprocal(out=rs[:], in_=sd[:])
                nc.vector.tensor_scalar(out=ht2[:, j, :], in0=xt[:, j, :],
                                        scalar1=mv[:, 0:1],
                                        scalar2=rs[:], op0=mybir.AluOpType.subtract,
                                        op1=mybir.AluOpType.mult)
            # one DMA xbar transpose per 256 tokens:
            # hT[p, i2*2+j, k, t] = ht2[t, j, k*128 + p]
            nc.sync.dma_start_transpose(
                hT[:, i2 * 2:(i2 + 1) * 2, :, :],
                ht2[:].rearrange("p i d -> p (i d)"))

    def qkv_units(hT, qT, kT, va, ntile, qkoff, with_q):
        """Generator: one QKV matmul group + copy per yield (PE filler units)."""
        ncol = ntile * 128
        # channel-major Q/K: psum [128 ch, ncol tokens]
        for ot in range(0 if with_q else 8, 16):
            pq = ps_mm.tile([128, 512], F32, tag="mm")
            for k in range(KT):
                nc.tensor.matmul(pq[:, 0:ncol], lhsT=wq_sbk[k][:, ts(ot, 128)],
                                 rhs=hT[:, 0:ntile, k, :], start=(k == 0), stop=(k == KT - 1))
            dest = qT[:, ot, qkoff:qkoff + ncol] if ot < 8 else kT[:, ot - 8, qkoff:qkoff + ncol]
            nc.scalar.activation(out=dest, in_=pq[:, 0:ncol],
                                 func=mybir.ActivationFunctionType.Copy,
                                 scale=sc_sb[:, ot:ot + 1])
            yield
        # token-major V: psum [128 tok, 512 ch]
        for i in range(ntile):
            for oh in range(2):
                pv = ps_mm.tile([128, 512], F32, tag="mm")
                for k in range(KT):
                    nc.tensor.matmul(pv[:], lhsT=hT[:, i, k, :],
                                     rhs=wq_sbk[k][:, 2048 + oh * 512: 2048 + (oh + 1) * 512],
                                     start=(k == 0), stop=(k == KT - 1))
                vt = (qkoff // 128) + i
                nc.scalar.copy(out=va[:, vt, oh * 8:(oh + 1) * 8, 0:64],
                               in_=pv[:].rearrange("p (h e) -> p h e", e=64))
                yield

    def outproj_units(attn_T, crow, irange=(0, 4)):
        for i in range(*irange):
            ot = op.tile([128, D], F32, tag="ot")
            for oh in range(2):
                po = ps_mm.tile([128, 512], F32, tag="mm")
                for k in range(KT):
                    nc.tensor.matmul(po[:], lhsT=attn_T[:, k, ts(i, 128)],
                                     rhs=wo_sbk[k][:, oh * 512:(oh + 1) * 512],
                                     start=(k == 0), stop=(k == KT - 1))
                nc.scalar.copy(out=ot[:, oh * 512:(oh + 1) * 512], in_=po[:])
                yield
            nc.sync.dma_start(ap_y[crow + i * 128: crow + (i + 1) * 128, :], ot[:])

    def attention_pairs(qT, kT_prev, kT_cur, va_prev, va_cur, attn_T, first_block):
        """Generator: per head-pair, scores+exp+mask staged one ahead of PV+norm.

        Emission order: sc(p) ... [yield] ... pv(p-1) sc(p+1) ... so the PE
        never head-of-line blocks on an exp that was just issued.
        """
        def units():
            for blk in range(2):
                qoff = blk * 256
                if blk == 0:
                    win = [(kT_prev, va_prev, 2), (kT_prev, va_prev, 3),
                           (kT_cur, va_cur, 0), (kT_cur, va_cur, 1)]
                else:
                    win = [(kT_cur, va_cur, 0), (kT_cur, va_cur, 1),
                           (kT_cur, va_cur, 2), (kT_cur, va_cur, 3)]
                midx = 0 if (first_block and blk == 0) else 1
                for hh in range(H // 2):
                    yield (blk, qoff, win, midx, hh)

        def emit_scores(u):
            blk, qoff, win, midx, hh = u
            exs = []
            for sub in range(2):
                p0 = sub * 64
                psc = ps_sc.tile([128, 4, 256], F32, tag="sc")
                for j, (kk, _, jt) in enumerate(win):
                    nc.tensor.matmul(psc[:, j, :],
                                     lhsT=kk[p0:p0 + 64, hh, ts(jt, 128)],
                                     rhs=qT[p0:p0 + 64, hh, qoff:qoff + 256],
                                     start=True, stop=True)
                ex = ep.tile([128, 4, 256], BF16, tag="ex")
                nc.scalar.activation(out=ex[:], in_=psc[:],
                                     func=mybir.ActivationFunctionType.Exp)
                exf = ex[:].rearrange("p a b -> p (a b)")
                meng = nc.vector if (sub == 0 or hh < 2) else nc.gpsimd
                meng.tensor_mul(out=exf, in0=exf, in1=mask_sb[:, midx, :])
                exs.append(ex)
            return exs

        def emit_pv(u, exs):
            blk, qoff, win, midx, hh = u
            ppv = ps_mm.tile([128, 512], F32, tag="pv")
            for sub in range(2):
                h = 2 * hh + sub
                col0 = sub * 256
                ex = exs[sub]
                for j, (_, vv, jt) in enumerate(win):
                    nc.tensor.matmul(ppv[0:65, col0:col0 + 256], lhsT=vv[:, jt, h, :],
                                     rhs=ex[:, j, :], start=(j == 0), stop=(j == 3))
            return ppv

        def emit_norm(u, ppv):
            blk, qoff, win, midx, hh = u
            # joint normalization for the head pair
            r16 = rp.tile([1, 512], BF16, tag="r16")
            with nc.allow_low_precision(reason="softmax denom reciprocal in bf16"):
                nc.vector.reciprocal(out=r16[:], in_=ppv[64:65, :])
            rbs = rp.tile([64, 512], BF16, tag="rbs")
            nc.gpsimd.partition_broadcast(rbs[:], r16[:])
            nc.vector.tensor_mul(out=attn_T[0:64, hh, qoff:qoff + 256],
                                 in0=ppv[0:64, 0:256], in1=rbs[:, 0:256])
            nc.vector.tensor_mul(out=attn_T[64:128, hh, qoff:qoff + 256],
                                 in0=ppv[0:64, 256:512], in1=rbs[:, 256:512])

        # stage PV two pairs behind scores (so exp+mask are done when PV
        # issues) and the DVE-side normalization one further pair behind
        # (so urgent masks aren't queued behind norm ops on the DVE).
        pend_sc = []
        pend_pv = []
        for u in units():
            exs = emit_scores(u)
            pend_sc.append((u, exs))
            if len(pend_sc) > 2:
                yield
                u2, exs2 = pend_sc.pop(0)
                pend_pv.append((u2, emit_pv(u2, exs2)))
            if len(pend_pv) > 1:
                emit_norm(*pend_pv.pop(0))
            yield
        while pend_sc:
            u2, exs2 = pend_sc.pop(0)
            pend_pv.append((u2, emit_pv(u2, exs2)))
            yield
        while pend_pv:
            emit_norm(*pend_pv.pop(0))

    def interleave(primary, fillers, late=None, late_from=22):
        """Alternate: one primary segment, then one filler unit. `late`
        fillers are only eligible from segment `late_from` (their deps are
        produced late in the primary stream)."""
        seg = 0
        done = False
        while not done:
            done = next(primary, "END") == "END"
            seg += 1
            if next(fillers, "END") == "END" and late is not None and seg >= late_from:
                next(late, None)
        for _ in fillers:
            pass
        if late is not None:
            for _ in late:
                pass

    def drain(gen):
        for _ in gen:
            pass

    # ---- prologue: halo tokens (local rows 0:256) -> kT/v tail positions ----
    def make_chunk_tensors(xts):
        hT = hTp.tile([128, 4, KT, 128], BF16, tag="hT")
        ln_transpose(hT, xts)
        qT = qTp.tile([128, 8, CHUNK], BF16, tag="qT")
        kT = kTp.tile([128, 8, CHUNK], BF16, tag="kT")
        va = vp.tile([128, 4, H, 65], BF16, tag="va")
        nc.vector.memset(va[:, :, :, 64:65], 1.0)
        return hT, qT, kT, va

    xts0h = load_x(0, 2)            # halo
    xts = {0: load_x(HALO, 4)}      # chunk 0
    hT0 = hTp.tile([128, 4, KT, 128], BF16, tag="hT")
    ln_transpose(hT0, xts0h)
    emit_weight_dmas()
    kT_prev = kTp.tile([128, 8, CHUNK], BF16, tag="kT")
    va_prev = vp.tile([128, 4, H, 65], BF16, tag="va")
    nc.vector.memset(va_prev[:, :, :, 64:65], 1.0)
    # chunk-0 LN emitted early: DVE/DMA work overlaps the weight DMAs and
    # the prologue QKV matmuls.
    hT, qT, kT, va = make_chunk_tensors(xts.pop(0))
    # chunk-0 QKV first (needs only x0 + wq); halo QKV fills in behind it
    gq0 = qkv_units(hT, qT, kT, va, 4, 0, with_q=True)
    for _ in range(8):
        next(gq0, None)
    xts[1] = load_x(HALO + CHUNK, 4)
    emit_wo_dmas()
    # halo goes to tail: kT_prev[:, :, 256:512], va tiles 2,3
    interleave(gq0, qkv_units(hT0, kT_prev, kT_prev, va_prev, 2, 256,
                              with_q=False))
    prev_outproj = None
    for c in range(NCHUNK):
        if c + 2 < NCHUNK:
            xts[c + 2] = load_x(HALO + (c + 2) * CHUNK, 4)
        attn_T = atp.tile([128, KT, CHUNK], BF16, tag="attn")
        attn = attention_pairs(qT, kT_prev, kT, va_prev, va, attn_T,
                               first_block=(c == 0))
        fillers = []
        if c + 1 < NCHUNK:
            hT2, qT2, kT2, va2 = make_chunk_tensors(xts.pop(c + 1))
            fillers.append(qkv_units(hT2, qT2, kT2, va2, 4, 0, with_q=True))
        if prev_outproj is not None:
            fillers.append(prev_outproj)

        def merged(gens):
            for g in gens:
                yield from g
        # first half of this chunk's outproj can fill the tail of its own
        # attention phase (block-0 tokens are fully normalized by then)
        interleave(attn, merged(fillers),
                   late=outproj_units(attn_T, c * CHUNK, (0, 2)))
        prev_outproj = outproj_units(attn_T, c * CHUNK, (2, 4))
        kT_prev, va_prev = kT, va
        if c + 1 < NCHUNK:
            hT, qT, kT, va = hT2, qT2, kT2, va2
    drain(prev_outproj)


def _prepare(x, norm_w, norm_b, qkv_w, out_w):
    wq_q, sc_qkv = _quantize(np.asarray(qkv_w, np.float32))
    wo_q, sc_out = _quantize(np.asarray(out_w, np.float32))
    g = np.asarray(norm_w, np.float32)
    b = np.asarray(norm_b, np.float32)
    if not np.allclose(g, 1.0):
        # fold the LN gain into the (no longer exactly ternary) qkv weight columns
        wq_q = wq_q * g[None, :]
    assert np.allclose(b, 0.0), "nonzero norm_b not supported"

    wqkvT = np.ascontiguousarray(wq_q.T).astype(ml_dtypes.bfloat16)  # [D, 3HD]
    # scales for q (with 1/sqrt(dh)) and k, applied on-device per output channel
    sc_qk = np.concatenate([sc_qkv[:1024] * (DH ** -0.5), sc_qkv[1024:2048]])
    sc_dev = sc_qk.reshape(16, 128).T.copy()  # [128, 16]
    # fold v-scale and out-scale into the output projection weight
    wout = wo_q * sc_out[:, None] * sc_qkv[None, 2048:3072]
    woutT = np.ascontiguousarray(wout.T).astype(ml_dtypes.bfloat16)  # [HD, D]

    m_first, m_rest = _build_masks()
    x = np.asarray(x, np.float32)
    in_maps = []
    for core in range(NCORES):
        bb, half = core // 2, core % 2
        xs = np.empty((NTOK, D), np.float32)
        if half == 0:
            xs[:HALO] = 0.0
            xs[HALO:] = x[bb, :OWN]
            masks = np.stack([m_first, m_rest])
        else:
            xs[:HALO] = x[bb, OWN - HALO:OWN]
            xs[HALO:] = x[bb, OWN:]
            masks = np.stack([m_rest, m_rest])
        in_maps.append({
            "x_shard": xs.astype(ml_dtypes.bfloat16),
            "wqkvT": wqkvT,
            "woutT": woutT,
            "sc_qk": sc_dev.astype(np.float32),
            "masks": np.ascontiguousarray(masks),
        })
    return in_maps


def get_nc():
    if "nc" not in _CACHE:
        _CACHE["nc"] = _build_bass()
    return _CACHE["nc"]


def run(in_maps, **kw):
    return run_bass_kernel_spmd(get_nc(), in_maps, core_ids=list(range(NCORES)), **kw)


def kernel(x, norm_w, norm_b, qkv_w, out_w):
    in_maps = _prepare(x, norm_w, norm_b, qkv_w, out_w)
    res = run(in_maps)
    y = np.empty((B, T, D), np.float32)
    for core in range(NCORES):
        bb, half = core // 2, core % 2
        y[bb, half * OWN:(half + 1) * OWN] = res.results[core]["y_shard"]
    return y
